# revision 11
# baseline (speedup 1.0000x reference)
"""CAM-module kernel for Trainium2, data-parallel over batch on 8 NeuronCores.

Per core (one batch sample, q = x[b] viewed as (C=512, N=4096) fp32):
  energy   = q @ q^T                      (C, C)   fp8 DoubleRow matmul, fp32 accum
  att[c,d] = softmax(max_d(energy) - energy)[c,d]
           = exp(m_c - e[c,d]) / Z_c      with m_c = row min of energy
  out      = gamma * (att @ q) + x

The row-max shift of the reference softmax cancels algebraically; only the
row minimum is needed for numerical stability (arguments of exp stay <= 0).

Structure:
 - x loads are issued as RAW DMAs before the TileContext (right after the
   semaphore-clear prologue + pseudo barrier), each bumping a dedicated
   semaphore on completion; the in-tile fp32->fp8 casts get explicit sem
   waits attached after tile scheduling.  This overlaps the framework
   prologue with the 8MB load.
 - All 128x128 transposes (q -> qT for energy, att -> attT for the output
   matmul) are plain fp8 matmuls against an identity moving operand: unlike
   PE transpose-mode these pipeline back-to-back and count as PE-busy for
   the HAM clock gate.
 - Energy is computed FULL-width (not upper-triangle + mirrored transposes):
   the DoubleRow energy matmuls are LDWEIGHTS-bound (256 weight columns per
   MM vs <=512 moving columns), so full-width rows cost the same PE time and
   remove the mirror transposes + copies and their serialization.
 - PSUM: 4 banks energy accumulators, 3 banks shared by the phase-A
   transpose staging and the phase-C att@q accumulators (disjoint in time),
   1 bank for attT transpose staging.  Keeping 3 dedicated po banks lets
   att@q matmuls run ahead of the epilogue adds instead of serializing
   MM -> add -> MM through a single bank.
 - The gamma/Z row scale rides in bias2 and is applied by the fused epilogue
   add (out = po * (gamma/Z_c) + x), split DVE (scalar_tensor_tensor) vs
   ACT-scale + GPSIMD-add.  Output is stored as fp16 (rel err ~5e-4, well
   under the 2e-2 gate) to halve the store traffic.
"""

import numpy as np

import concourse.bass as bass
import concourse.tile as tile
from concourse import mybir
from concourse.masks import make_identity
from concourse.vector_clock import ScopedClock

P = 128
C = 512
N = 4096
B = 8
CT = C // P   # 4 c-tiles
KT = N // P   # 32 n-chunks of 128
GG = 4        # 1024-col load super-groups

STRIP_TAIL = True

FP32 = mybir.dt.float32
FP16 = mybir.dt.float16
FP8 = mybir.dt.float8e4
DR = mybir.MatmulPerfMode.DoubleRow
MIN = mybir.AluOpType.min


def _drain_and_barrier_split(self, tick_clock, wait_clock):
    # The pinned walrus rejects >1 sync-wait on TPB_CTRL (Drain); spread the
    # final global-clock waits across a chain of drains, one wait each.
    nc = self.nc
    drain_inst = nc.sync.drain()
    wait_clock.add_sem_waits(
        drain_inst.ins, ScopedClock({None: tick_clock.global_clock})
    )
    si = drain_inst.ins.sync_info
    if si is not None and si.on_wait is not None and len(si.on_wait) > 1:
        waits = list(si.on_wait)
        si.on_wait = waits[:1]
        for w in waits[1:]:
            extra = nc.sync.drain()
            extra.ins.sync_info = mybir.SyncInfo(on_wait=[w], on_update=[])
    nc.all_engine_barrier()
    assert self.sems is not None
    popped = nc._tile_sem_poison_stack.pop()
    assert popped is self._sem_poison
    if not STRIP_TAIL:
        nc.clear_and_free_semaphores(list(self.sems.allocated().values()))
        nc.all_engine_barrier()


tile.TileContext._drain_and_barrier = _drain_and_barrier_split


def _legalize_sync_waits(nc):
    # This walrus build rejects instructions carrying more than one sync-wait.
    # Hoist extra waits onto same-engine NoOps placed immediately before the
    # instruction (engine streams preserve relative order within a block).
    for f in nc.m.functions:
        for bb in f.blocks:
            new = []
            for inst in bb.instructions:
                si = inst.sync_info
                if si is not None and si.on_wait and len(si.on_wait) > 1:
                    waits = list(si.on_wait)
                    for w in waits[:-1]:
                        nop = mybir.InstNoOp(
                            name=nc.get_next_instruction_name(),
                            engine=inst.engine,
                            bass_nofuse=True,
                            sync_info=mybir.SyncInfo(on_wait=[w], on_update=[]),
                        )
                        new.append(nop)
                    si.on_wait = [waits[-1]]
                new.append(inst)
            bb.instructions[:] = new


def build_nc():
    nc = bass.Bass()
    x_d = nc.declare_dram_parameter("x", [C, N], FP32, isOutput=False)
    g_d = nc.declare_dram_parameter("gamma", [1, 1], FP32, isOutput=False)
    o_d = nc.declare_dram_parameter("out", [C, N], FP16, isOutput=True)

    # Clear kernel semaphores at START (idle window) instead of paying the
    # expensive teardown clear+barrier at the end (STRIP_TAIL above).  The
    # x-load semaphores below are inside the cleared range, so a re-run of
    # the NEFF starts from zero again.
    from concourse.bass import compact_to_ranges

    for sem_range in compact_to_ranges(
        [sem for sem in nc._kernel_sem_range if sem not in nc.barrier_sems]
    ):
        nc.gpsimd.sem_clear(sem_range)
    nc._nrt_pseudo_barrier()

    # Early x loads: raw DMAs issued before the TileContext so the 8MB input
    # streams during the framework prologue.  One [128,1024] chunk per
    # (super-group gg, row-block ci), one completion semaphore each; issue
    # engines are spread so every gg=0 chunk is issued within ~700ns.
    xf = [nc.alloc_sbuf_tensor(f"xraw{ci}", [P, N], FP32) for ci in range(CT)]
    xsem = [[nc.alloc_semaphore(f"xld{gg}_{ci}") for ci in range(CT)] for gg in range(GG)]
    issue_plan = {
        0: [(0, 0), (0, 1), (1, 0), (1, 1), (2, 0), (2, 1), (3, 0)],   # sync
        1: [(0, 2), (1, 2), (2, 2), (3, 1), (3, 2)],                   # gpsimd
        2: [(0, 3), (1, 3), (2, 3), (3, 3)],                           # scalar
    }
    engs = [nc.sync, nc.gpsimd, nc.scalar]
    for ei, plan in issue_plan.items():
        for gg, ci in plan:
            rows = slice(ci * P, (ci + 1) * P)
            cols = slice(gg * 1024, (gg + 1) * 1024)
            engs[ei].dma_start(out=xf[ci][:, cols], in_=x_d[rows, cols]).then_inc(
                xsem[gg][ci], 16
            )

    cast_waits = []  # (BassInstruction, gg, ci): xsem waits attached post-scheduling

    with tile.TileContext(nc) as tc:
        with (
            tc.tile_pool(name="singles", bufs=1) as singles,
            tc.tile_pool(name="stage", bufs=4) as stage,
            tc.tile_pool(name="psum_acc", bufs=4, space="PSUM") as psum_acc,
            tc.tile_pool(name="psum_po", bufs=3, space="PSUM") as psum_po,
            tc.tile_pool(name="psum_ptr", bufs=1, space="PSUM") as psum_ptr,
        ):
            # PE warm-up on a dep-free tile, ACT Exp-table preload on a dummy,
            # identity before the gamma broadcast DMA.
            warm8 = singles.tile([P, P], FP8, tag="warm8")
            nc.vector.memset(warm8[:], 1.0)
            for _ in range(20):
                wp = psum_po.tile([P, 512], FP32, tag="po")
                nc.tensor.matmul(
                    wp[:, 0:P], lhsT=warm8[:], rhs=warm8[:], start=True, stop=True
                )
            dume = singles.tile([P, 1], FP32, tag="dume")
            nc.scalar.activation(
                out=dume[:], in_=warm8[:, 0:1], func=mybir.ActivationFunctionType.Exp
            )
            id8 = singles.tile([P, P], FP8, tag="id8")
            make_identity(nc, id8)
            gcol = singles.tile([P, 1], FP32, tag="gamma")
            nc.gpsimd.dma_start(out=gcol[:], in_=g_d[:, :].to_broadcast((P, 1)))

            q8 = singles.tile([P, CT, N], FP8, tag="q8")
            qT = singles.tile([P, KT, 512], FP8, tag="qT")
            e_ps = [
                psum_acc.tile([P, 512], FP32, tag="acc", name=f"e{ci}")
                for ci in range(CT)
            ]

            # Phase A: per super-group, cast to fp8 (ACT + GPSIMD, gated on
            # the raw load sems), transpose 128x128 blocks into qT (plain fp8
            # matmul vs identity -> PSUM fp32 -> DVE copy), accumulate
            # full-width DoubleRow energy matmuls (pairs of 128-chunks).
            for gg in range(GG):
                base = gg * 1024
                for ci in range(CT):
                    # GPSIMD casts the first supergroups of rows 2-3 (its own
                    # loaded chunks); ACT casts the rest.
                    use_gp = gg <= 1 and ci >= 2
                    for half in range(2):
                        src = xf[ci][:, base + half * 512 : base + (half + 1) * 512]
                        dst = q8[:, ci, base + half * 512 : base + (half + 1) * 512]
                        if use_gp:
                            cst = nc.gpsimd.tensor_copy(out=dst, in_=src)
                        else:
                            cst = nc.scalar.copy(out=dst, in_=src)
                        cast_waits.append((cst, gg, ci))
                for tt in range(4):  # pairs of 128-chunks within super-group
                    t = gg * 4 + tt
                    for k in (2 * t, 2 * t + 1):
                        pt = psum_po.tile([P, 512], FP32, tag="po")
                        for ci in range(CT):
                            nc.tensor.matmul(
                                pt[:, ci * P : (ci + 1) * P],
                                lhsT=q8[:, ci, k * P : (k + 1) * P],
                                rhs=id8[:],
                                start=True,
                                stop=True,
                            )
                        nc.vector.tensor_copy(out=qT[:, k, :], in_=pt[:])
                    for ci in range(CT):
                        nc.tensor.matmul(
                            e_ps[ci][:],
                            lhsT=qT[:, 2 * t : 2 * t + 2, ci * P : (ci + 1) * P],
                            rhs=qT[:, 2 * t : 2 * t + 2, :],
                            start=(t == 0),
                            stop=(t == KT // 2 - 1),
                            perf_mode=DR,
                        )

            # Softmax per ci: row min (DVE), exp with fp8 out + row-sum
            # accumulator (ACT), 1/Z and gamma/Z (DVE small).  attT via plain
            # fp8 matmul transposes + scalar copies into EXPT.  The gamma/Z
            # row scale is NOT applied to EXPQ; it rides in bias2 and is
            # applied by the epilogue adds.  ci=0 is the only chain on the
            # critical path (the others hide under att@q), so its min/exp run
            # in halves to shorten the serial chain.
            mcol = singles.tile([P, CT], FP32, tag="m")
            mh = singles.tile([P, 2], FP32, tag="mh")
            zcol = singles.tile([P, CT], FP32, tag="z")
            zh = singles.tile([P, 2], FP32, tag="zh")
            lnz = singles.tile([P, CT], FP32, tag="lnz")
            bias2 = singles.tile([P, CT], FP32, tag="bias2")
            EXPQ = singles.tile([P, CT, 512], FP8, tag="EXPQ")
            EXPT = singles.tile([P, CT, 512], FP8, tag="EXPT")

            def softmax_head(ci):
                cs = slice(ci, ci + 1)
                if ci == 0:
                    nc.vector.tensor_reduce(
                        out=mh[:, 0:1], in_=e_ps[0][:, 0:256],
                        axis=mybir.AxisListType.X, op=MIN,
                    )
                    nc.vector.tensor_reduce(
                        out=mh[:, 1:2], in_=e_ps[0][:, 256:512],
                        axis=mybir.AxisListType.X, op=MIN,
                    )
                    nc.vector.tensor_tensor(
                        out=mcol[:, 0:1], in0=mh[:, 0:1], in1=mh[:, 1:2], op=MIN
                    )
                else:
                    nc.vector.tensor_reduce(
                        out=mcol[:, cs], in_=e_ps[ci][:],
                        axis=mybir.AxisListType.X, op=MIN,
                    )

            def softmax_tail(ci):
                cs = slice(ci, ci + 1)
                if ci == 0:
                    for half in range(2):
                        nc.scalar.activation(
                            out=EXPQ[:, 0, half * 256 : (half + 1) * 256],
                            in_=e_ps[0][:, half * 256 : (half + 1) * 256],
                            func=mybir.ActivationFunctionType.Exp,
                            bias=mcol[:, 0:1],
                            scale=-1.0,
                            accum_out=zh[:, half : half + 1],
                        )
                    nc.vector.tensor_add(
                        out=zcol[:, 0:1], in0=zh[:, 0:1], in1=zh[:, 1:2]
                    )
                else:
                    nc.scalar.activation(
                        out=EXPQ[:, ci, :],
                        in_=e_ps[ci][:],
                        func=mybir.ActivationFunctionType.Exp,
                        bias=mcol[:, cs],
                        scale=-1.0,
                        accum_out=zcol[:, cs],
                    )
                nc.vector.reciprocal(out=lnz[:, cs], in_=zcol[:, cs])
                nc.vector.tensor_mul(out=bias2[:, cs], in0=lnz[:, cs], in1=gcol[:])
                for dj in range(CT):
                    ptx = psum_ptr.tile([P, P], FP32, tag="ptr")
                    nc.tensor.matmul(
                        ptx[:],
                        lhsT=EXPQ[:, ci, dj * P : (dj + 1) * P],
                        rhs=id8[:],
                        start=True,
                        stop=True,
                    )
                    nc.scalar.copy(
                        out=EXPT[:, dj, ci * P : (ci + 1) * P], in_=ptx[:]
                    )

            def attq(ci):
                # att@q (DoubleRow, K=512 via two K=256 groups) + fused
                # epilogue add out = po * (gamma/Z_c) + x, fp16 store.
                for nh in range(2):
                    osb = stage.tile([P, 2048], FP16, tag="osb")
                    for sub in range(4):
                        nj = nh * 4 + sub
                        po = psum_po.tile([P, 512], FP32, tag="po")
                        for j in range(2):
                            nc.tensor.matmul(
                                po[:],
                                lhsT=EXPT[:, 2 * j : 2 * j + 2, ci * P : (ci + 1) * P],
                                rhs=q8[:, 2 * j : 2 * j + 2, nj * 512 : (nj + 1) * 512],
                                start=(j == 0),
                                stop=(j == 1),
                                perf_mode=DR,
                            )
                        if sub < 3:
                            nc.vector.scalar_tensor_tensor(
                                out=osb[:, sub * 512 : (sub + 1) * 512],
                                in0=po[:],
                                scalar=bias2[:, ci : ci + 1],
                                in1=xf[ci][:, nj * 512 : (nj + 1) * 512],
                                op0=mybir.AluOpType.mult,
                                op1=mybir.AluOpType.add,
                            )
                        else:
                            tmp = stage.tile([P, 512], FP32, tag="tmp")
                            nc.scalar.mul(
                                out=tmp[:], in_=po[:], mul=bias2[:, ci : ci + 1]
                            )
                            nc.gpsimd.tensor_add(
                                out=osb[:, sub * 512 : (sub + 1) * 512],
                                in0=tmp[:],
                                in1=xf[ci][:, nj * 512 : (nj + 1) * 512],
                            )
                    nc.sync.dma_start(
                        out=o_d[ci * P : (ci + 1) * P, nh * 2048 : (nh + 1) * 2048],
                        in_=osb[:],
                    )

            for ci in range(CT):
                softmax_head(ci)
            for ci in range(CT):
                softmax_tail(ci)
                attq(ci)

    # The raw-load gating is invisible to the tile scheduler (its deadlock
    # simulator would stall on semaphores no in-context instruction bumps),
    # so attach the waits only after scheduling has run.
    for cst, gg, ci in cast_waits:
        cst.wait_op(xsem[gg][ci], 16, "sem-ge")
    _legalize_sync_waits(nc)
    return nc


def make_in_maps(x, gamma):
    x = np.ascontiguousarray(np.asarray(x, dtype=np.float32)).reshape(B, C, N)
    g = np.ascontiguousarray(np.asarray(gamma, dtype=np.float32)).reshape(1, 1)
    return [{"x": x[i], "gamma": g} for i in range(B)]


def kernel(x, y=None, gamma=None, **_ignored):
    from concourse.bass_utils import run_bass_kernel_spmd

    nc = build_nc()
    in_maps = make_in_maps(x, gamma)
    res = run_bass_kernel_spmd(nc, in_maps, list(range(B)))
    out = np.stack([np.asarray(res.results[i]["out"]) for i in range(B)])
    return out.reshape(B, C, 64, 64).astype(np.float32)


# revision 14
# speedup vs baseline: 1.0042x; 1.0042x over previous
"""CAM-module kernel for Trainium2, data-parallel over batch on 8 NeuronCores.

Per core (one batch sample, q = x[b] viewed as (C=512, N=4096) fp32):
  energy   = q @ q^T                      (C, C)   fp8 DoubleRow matmul, fp32 accum
  att[c,d] = softmax(max_d(energy) - energy)[c,d]
           = exp(m_c - e[c,d]) / Z_c      with m_c = row min of energy
  out      = gamma * (att @ q) + x

The row-max shift of the reference softmax cancels algebraically; only the
row minimum is needed for numerical stability (arguments of exp stay <= 0).

Structure:
 - x loads are issued as RAW DMAs before the TileContext (right after the
   semaphore-clear prologue + pseudo barrier), each bumping a dedicated
   semaphore on completion; the in-tile fp32->fp8 casts get explicit sem
   waits attached after tile scheduling.  This overlaps the framework
   prologue with the 8MB load.
 - All 128x128 transposes (q -> qT for energy, att -> attT for the output
   matmul) are plain fp8 matmuls against an identity moving operand: unlike
   PE transpose-mode these pipeline back-to-back and count as PE-busy for
   the HAM clock gate.
 - Energy is computed FULL-width (not upper-triangle + mirrored transposes):
   the DoubleRow energy matmuls are LDWEIGHTS-bound (256 weight columns per
   MM vs <=512 moving columns), so full-width rows cost the same PE time and
   remove the mirror transposes + copies and their serialization.
 - PSUM: 4 banks energy accumulators, 3 banks shared by the phase-A
   transpose staging and the phase-C att@q accumulators (disjoint in time),
   1 bank for attT transpose staging.  Keeping 3 dedicated po banks lets
   att@q matmuls run ahead of the epilogue adds instead of serializing
   MM -> add -> MM through a single bank.
 - The gamma/Z row scale rides in bias2 and is applied by the fused epilogue
   add (out = po * (gamma/Z_c) + x), split DVE (scalar_tensor_tensor) vs
   ACT-scale + GPSIMD-add.  Output is stored as fp16 (rel err ~5e-4, well
   under the 2e-2 gate) to halve the store traffic.
"""

import numpy as np

import concourse.bass as bass
import concourse.tile as tile
from concourse import mybir
from concourse.masks import make_identity
from concourse.vector_clock import ScopedClock

P = 128
C = 512
N = 4096
B = 8
CT = C // P   # 4 c-tiles
KT = N // P   # 32 n-chunks of 128
GG = 4        # 1024-col load super-groups

STRIP_TAIL = True

FP32 = mybir.dt.float32
FP16 = mybir.dt.float16
FP8 = mybir.dt.float8e4
DR = mybir.MatmulPerfMode.DoubleRow
MIN = mybir.AluOpType.min


def _drain_and_barrier_split(self, tick_clock, wait_clock):
    # The pinned walrus rejects >1 sync-wait on TPB_CTRL (Drain); spread the
    # final global-clock waits across a chain of drains, one wait each.
    nc = self.nc
    drain_inst = nc.sync.drain()
    wait_clock.add_sem_waits(
        drain_inst.ins, ScopedClock({None: tick_clock.global_clock})
    )
    si = drain_inst.ins.sync_info
    if si is not None and si.on_wait is not None and len(si.on_wait) > 1:
        waits = list(si.on_wait)
        si.on_wait = waits[:1]
        for w in waits[1:]:
            extra = nc.sync.drain()
            extra.ins.sync_info = mybir.SyncInfo(on_wait=[w], on_update=[])
    nc.all_engine_barrier()
    assert self.sems is not None
    popped = nc._tile_sem_poison_stack.pop()
    assert popped is self._sem_poison
    if not STRIP_TAIL:
        nc.clear_and_free_semaphores(list(self.sems.allocated().values()))
        nc.all_engine_barrier()


tile.TileContext._drain_and_barrier = _drain_and_barrier_split


def _legalize_sync_waits(nc):
    # This walrus build rejects instructions carrying more than one sync-wait.
    # Hoist extra waits onto same-engine NoOps placed immediately before the
    # instruction (engine streams preserve relative order within a block).
    for f in nc.m.functions:
        for bb in f.blocks:
            new = []
            for inst in bb.instructions:
                si = inst.sync_info
                if si is not None and si.on_wait and len(si.on_wait) > 1:
                    waits = list(si.on_wait)
                    for w in waits[:-1]:
                        nop = mybir.InstNoOp(
                            name=nc.get_next_instruction_name(),
                            engine=inst.engine,
                            bass_nofuse=True,
                            sync_info=mybir.SyncInfo(on_wait=[w], on_update=[]),
                        )
                        new.append(nop)
                    si.on_wait = [waits[-1]]
                new.append(inst)
            bb.instructions[:] = new


def build_nc():
    nc = bass.Bass()
    x_d = nc.declare_dram_parameter("x", [C, N], FP32, isOutput=False)
    g_d = nc.declare_dram_parameter("gamma", [1, 1], FP32, isOutput=False)
    o_d = nc.declare_dram_parameter("out", [C, N], FP16, isOutput=True)

    # Clear kernel semaphores at START (idle window) instead of paying the
    # expensive teardown clear+barrier at the end (STRIP_TAIL above).  The
    # x-load semaphores below are inside the cleared range, so a re-run of
    # the NEFF starts from zero again.  (Dropping this hangs the device —
    # semaphore state persists across NEFF loads.)
    from concourse.bass import compact_to_ranges

    for sem_range in compact_to_ranges(
        [sem for sem in nc._kernel_sem_range if sem not in nc.barrier_sems]
    ):
        nc.gpsimd.sem_clear(sem_range)
    nc._nrt_pseudo_barrier()

    # Early x loads: raw DMAs issued before the TileContext so the 8MB input
    # streams during the framework prologue.  One [128,1024] chunk per
    # (super-group gg, row-block ci), one completion semaphore each; issue
    # engines are spread so every gg=0 chunk is issued within ~700ns.
    xf = [nc.alloc_sbuf_tensor(f"xraw{ci}", [P, N], FP32) for ci in range(CT)]
    xsem = [[nc.alloc_semaphore(f"xld{gg}_{ci}") for ci in range(CT)] for gg in range(GG)]
    issue_plan = {
        0: [(0, 0), (0, 1), (1, 0), (1, 1), (2, 0), (2, 1), (3, 0)],   # sync
        1: [(0, 2), (1, 2), (2, 2), (3, 1), (3, 2)],                   # gpsimd
        2: [(0, 3), (1, 3), (2, 3), (3, 3)],                           # scalar
    }
    engs = [nc.sync, nc.gpsimd, nc.scalar]
    for ei, plan in issue_plan.items():
        for gg, ci in plan:
            rows = slice(ci * P, (ci + 1) * P)
            cols = slice(gg * 1024, (gg + 1) * 1024)
            engs[ei].dma_start(out=xf[ci][:, cols], in_=x_d[rows, cols]).then_inc(
                xsem[gg][ci], 16
            )

    cast_waits = []  # (BassInstruction, gg, ci): xsem waits attached post-scheduling

    with tile.TileContext(nc) as tc:
        with (
            tc.tile_pool(name="singles", bufs=1) as singles,
            tc.tile_pool(name="stage", bufs=4) as stage,
            tc.tile_pool(name="psum_acc", bufs=4, space="PSUM") as psum_acc,
            tc.tile_pool(name="psum_po", bufs=3, space="PSUM") as psum_po,
            tc.tile_pool(name="psum_ptr", bufs=1, space="PSUM") as psum_ptr,
        ):
            # PE warm-up on a dep-free tile, ACT Exp-table preload on a dummy,
            # identity before the gamma broadcast DMA.
            warm8 = singles.tile([P, P], FP8, tag="warm8")
            nc.vector.memset(warm8[:], 1.0)
            for _ in range(20):
                wp = psum_po.tile([P, 512], FP32, tag="po")
                nc.tensor.matmul(
                    wp[:, 0:P], lhsT=warm8[:], rhs=warm8[:], start=True, stop=True
                )
            dume = singles.tile([P, 1], FP32, tag="dume")
            nc.scalar.activation(
                out=dume[:], in_=warm8[:, 0:1], func=mybir.ActivationFunctionType.Exp
            )
            id8 = singles.tile([P, P], FP8, tag="id8")
            make_identity(nc, id8)
            gcol = singles.tile([P, 1], FP32, tag="gamma")
            nc.gpsimd.dma_start(out=gcol[:], in_=g_d[:, :].to_broadcast((P, 1)))

            q8 = singles.tile([P, CT, N], FP8, tag="q8")
            qT = singles.tile([P, KT, 512], FP8, tag="qT")
            e_ps = [
                psum_acc.tile([P, 512], FP32, tag="acc", name=f"e{ci}")
                for ci in range(CT)
            ]

            # Phase A: per super-group, cast to fp8 (ACT + GPSIMD, gated on
            # the raw load sems), transpose 128x128 blocks into qT (plain fp8
            # matmul vs identity -> PSUM fp32 -> DVE copy), accumulate
            # full-width DoubleRow energy matmuls (pairs of 128-chunks).
            for gg in range(GG):
                base = gg * 1024
                for ci in range(CT):
                    # GPSIMD (slow: ~1.9us per half-cast) only assists on two
                    # mid-stream chunks to keep ACT's queue from falling
                    # behind the load stream; everything latency-critical
                    # stays on ACT (~0.71us per half-cast).
                    use_gp = ci == 3 and gg in (1, 2)
                    for half in range(2):
                        src = xf[ci][:, base + half * 512 : base + (half + 1) * 512]
                        dst = q8[:, ci, base + half * 512 : base + (half + 1) * 512]
                        if use_gp:
                            cst = nc.gpsimd.tensor_copy(out=dst, in_=src)
                        else:
                            cst = nc.scalar.copy(out=dst, in_=src)
                        cast_waits.append((cst, gg, ci))
                for tt in range(4):  # pairs of 128-chunks within super-group
                    t = gg * 4 + tt
                    for k in (2 * t, 2 * t + 1):
                        pt = psum_po.tile([P, 512], FP32, tag="po")
                        for ci in range(CT):
                            nc.tensor.matmul(
                                pt[:, ci * P : (ci + 1) * P],
                                lhsT=q8[:, ci, k * P : (k + 1) * P],
                                rhs=id8[:],
                                start=True,
                                stop=True,
                            )
                        nc.vector.tensor_copy(out=qT[:, k, :], in_=pt[:])
                    for ci in range(CT):
                        nc.tensor.matmul(
                            e_ps[ci][:],
                            lhsT=qT[:, 2 * t : 2 * t + 2, ci * P : (ci + 1) * P],
                            rhs=qT[:, 2 * t : 2 * t + 2, :],
                            start=(t == 0),
                            stop=(t == KT // 2 - 1),
                            perf_mode=DR,
                        )

            # Softmax per ci: row min (DVE), exp with fp8 out + row-sum
            # accumulator (ACT), 1/Z and gamma/Z (DVE small).  attT via plain
            # fp8 matmul transposes + scalar copies into EXPT.  The gamma/Z
            # row scale is NOT applied to EXPQ; it rides in bias2 and is
            # applied by the epilogue adds.  ci=0 is the only chain on the
            # critical path (the others hide under att@q), so its min/exp run
            # in halves to shorten the serial chain.
            mcol = singles.tile([P, CT], FP32, tag="m")
            mh = singles.tile([P, 2], FP32, tag="mh")
            zcol = singles.tile([P, CT], FP32, tag="z")
            zh = singles.tile([P, 2], FP32, tag="zh")
            lnz = singles.tile([P, CT], FP32, tag="lnz")
            bias2 = singles.tile([P, CT], FP32, tag="bias2")
            EXPQ = singles.tile([P, CT, 512], FP8, tag="EXPQ")
            EXPT = singles.tile([P, CT, 512], FP8, tag="EXPT")

            def softmax_head(ci):
                cs = slice(ci, ci + 1)
                if ci == 0:
                    nc.vector.tensor_reduce(
                        out=mh[:, 0:1], in_=e_ps[0][:, 0:256],
                        axis=mybir.AxisListType.X, op=MIN,
                    )
                    nc.vector.tensor_reduce(
                        out=mh[:, 1:2], in_=e_ps[0][:, 256:512],
                        axis=mybir.AxisListType.X, op=MIN,
                    )
                    nc.vector.tensor_tensor(
                        out=mcol[:, 0:1], in0=mh[:, 0:1], in1=mh[:, 1:2], op=MIN
                    )
                else:
                    nc.vector.tensor_reduce(
                        out=mcol[:, cs], in_=e_ps[ci][:],
                        axis=mybir.AxisListType.X, op=MIN,
                    )

            def softmax_tail(ci):
                cs = slice(ci, ci + 1)
                if ci == 0:
                    for half in range(2):
                        nc.scalar.activation(
                            out=EXPQ[:, 0, half * 256 : (half + 1) * 256],
                            in_=e_ps[0][:, half * 256 : (half + 1) * 256],
                            func=mybir.ActivationFunctionType.Exp,
                            bias=mcol[:, 0:1],
                            scale=-1.0,
                            accum_out=zh[:, half : half + 1],
                        )
                    nc.vector.tensor_add(
                        out=zcol[:, 0:1], in0=zh[:, 0:1], in1=zh[:, 1:2]
                    )
                else:
                    nc.scalar.activation(
                        out=EXPQ[:, ci, :],
                        in_=e_ps[ci][:],
                        func=mybir.ActivationFunctionType.Exp,
                        bias=mcol[:, cs],
                        scale=-1.0,
                        accum_out=zcol[:, cs],
                    )
                nc.vector.reciprocal(out=lnz[:, cs], in_=zcol[:, cs])
                nc.vector.tensor_mul(out=bias2[:, cs], in0=lnz[:, cs], in1=gcol[:])
                for dj in range(CT):
                    ptx = psum_ptr.tile([P, P], FP32, tag="ptr")
                    nc.tensor.matmul(
                        ptx[:],
                        lhsT=EXPQ[:, ci, dj * P : (dj + 1) * P],
                        rhs=id8[:],
                        start=True,
                        stop=True,
                    )
                    nc.scalar.copy(
                        out=EXPT[:, dj, ci * P : (ci + 1) * P], in_=ptx[:]
                    )

            def attq(ci):
                # att@q (DoubleRow, K=512 via two K=256 groups) + fused
                # epilogue add out = po * (gamma/Z_c) + x, fp16 store.
                for nh in range(2):
                    osb = stage.tile([P, 2048], FP16, tag="osb")
                    for sub in range(4):
                        nj = nh * 4 + sub
                        po = psum_po.tile([P, 512], FP32, tag="po")
                        for j in range(2):
                            nc.tensor.matmul(
                                po[:],
                                lhsT=EXPT[:, 2 * j : 2 * j + 2, ci * P : (ci + 1) * P],
                                rhs=q8[:, 2 * j : 2 * j + 2, nj * 512 : (nj + 1) * 512],
                                start=(j == 0),
                                stop=(j == 1),
                                perf_mode=DR,
                            )
                        if sub < 3:
                            nc.vector.scalar_tensor_tensor(
                                out=osb[:, sub * 512 : (sub + 1) * 512],
                                in0=po[:],
                                scalar=bias2[:, ci : ci + 1],
                                in1=xf[ci][:, nj * 512 : (nj + 1) * 512],
                                op0=mybir.AluOpType.mult,
                                op1=mybir.AluOpType.add,
                            )
                        else:
                            tmp = stage.tile([P, 512], FP32, tag="tmp")
                            nc.scalar.mul(
                                out=tmp[:], in_=po[:], mul=bias2[:, ci : ci + 1]
                            )
                            nc.gpsimd.tensor_add(
                                out=osb[:, sub * 512 : (sub + 1) * 512],
                                in0=tmp[:],
                                in1=xf[ci][:, nj * 512 : (nj + 1) * 512],
                            )
                    nc.sync.dma_start(
                        out=o_d[ci * P : (ci + 1) * P, nh * 2048 : (nh + 1) * 2048],
                        in_=osb[:],
                    )

            for ci in range(CT):
                softmax_head(ci)
            for ci in range(CT):
                softmax_tail(ci)
                attq(ci)

    # The raw-load gating is invisible to the tile scheduler (its deadlock
    # simulator would stall on semaphores no in-context instruction bumps),
    # so attach the waits only after scheduling has run.
    for cst, gg, ci in cast_waits:
        cst.wait_op(xsem[gg][ci], 16, "sem-ge")
    _legalize_sync_waits(nc)
    return nc


def make_in_maps(x, gamma):
    x = np.ascontiguousarray(np.asarray(x, dtype=np.float32)).reshape(B, C, N)
    g = np.ascontiguousarray(np.asarray(gamma, dtype=np.float32)).reshape(1, 1)
    return [{"x": x[i], "gamma": g} for i in range(B)]


def kernel(x, y=None, gamma=None, **_ignored):
    from concourse.bass_utils import run_bass_kernel_spmd

    nc = build_nc()
    in_maps = make_in_maps(x, gamma)
    res = run_bass_kernel_spmd(nc, in_maps, list(range(B)))
    out = np.stack([np.asarray(res.results[i]["out"]) for i in range(B)])
    return out.reshape(B, C, 64, 64).astype(np.float32)


# revision 16
# speedup vs baseline: 1.0619x; 1.0575x over previous
"""CAM-module kernel for Trainium2, data-parallel over batch on 8 NeuronCores.

Per core (one batch sample, q = x[b] viewed as (C=512, N=4096) fp32):
  energy   = q @ q^T                      (C, C)   fp8 DoubleRow matmul, fp32 accum
  att[c,d] = softmax(max_d(energy) - energy)[c,d]
           = exp(m_c - e[c,d]) / Z_c      with m_c = row min of energy
  out      = gamma * (att @ q) + x

The row-max shift of the reference softmax cancels algebraically; only the
row minimum is needed for numerical stability (arguments of exp stay <= 0).

Structure:
 - x loads are issued as RAW DMAs before the TileContext (right after the
   semaphore-clear prologue + pseudo barrier), each bumping a dedicated
   semaphore on completion; the in-tile fp32->fp8 casts get explicit sem
   waits attached after tile scheduling.  This overlaps the framework
   prologue with the 8MB load.
 - All 128x128 transposes (q -> qT for energy, att -> attT for the output
   matmul) are plain fp8 matmuls against an identity moving operand: unlike
   PE transpose-mode these pipeline back-to-back and count as PE-busy for
   the HAM clock gate.
 - Energy is computed FULL-width (not upper-triangle + mirrored transposes):
   the DoubleRow energy matmuls are LDWEIGHTS-bound (256 weight columns per
   MM vs <=512 moving columns), so full-width rows cost the same PE time and
   remove the mirror transposes + copies and their serialization.
 - PSUM: 4 banks energy accumulators, 3 banks shared by the phase-A
   transpose staging and the phase-C att@q accumulators (disjoint in time),
   1 bank for attT transpose staging.  Keeping 3 dedicated po banks lets
   att@q matmuls run ahead of the epilogue adds instead of serializing
   MM -> add -> MM through a single bank.
 - The gamma/Z row scale rides in bias2 and is applied by the fused epilogue
   add (out = po * (gamma/Z_c) + x), split DVE (scalar_tensor_tensor) vs
   ACT-scale + GPSIMD-add.  Output is stored as fp16 (rel err ~5e-4, well
   under the 2e-2 gate) to halve the store traffic.
"""

import numpy as np

import concourse.bass as bass
import concourse.tile as tile
from concourse import mybir
from concourse.masks import make_identity
from concourse.vector_clock import ScopedClock

P = 128
C = 512
N = 4096
B = 8
CT = C // P   # 4 c-tiles
KT = N // P   # 32 n-chunks of 128
GG = 4        # 1024-col load super-groups

STRIP_TAIL = True

FP32 = mybir.dt.float32
FP16 = mybir.dt.float16
FP8 = mybir.dt.float8e4
DR = mybir.MatmulPerfMode.DoubleRow
MIN = mybir.AluOpType.min


def _drain_and_barrier_split(self, tick_clock, wait_clock):
    # The pinned walrus rejects >1 sync-wait on TPB_CTRL (Drain); spread the
    # final global-clock waits across a chain of drains, one wait each.
    nc = self.nc
    drain_inst = nc.sync.drain()
    wait_clock.add_sem_waits(
        drain_inst.ins, ScopedClock({None: tick_clock.global_clock})
    )
    si = drain_inst.ins.sync_info
    if si is not None and si.on_wait is not None and len(si.on_wait) > 1:
        waits = list(si.on_wait)
        si.on_wait = waits[:1]
        for w in waits[1:]:
            extra = nc.sync.drain()
            extra.ins.sync_info = mybir.SyncInfo(on_wait=[w], on_update=[])
    nc.all_engine_barrier()
    assert self.sems is not None
    popped = nc._tile_sem_poison_stack.pop()
    assert popped is self._sem_poison
    if not STRIP_TAIL:
        nc.clear_and_free_semaphores(list(self.sems.allocated().values()))
        nc.all_engine_barrier()


tile.TileContext._drain_and_barrier = _drain_and_barrier_split


def _legalize_sync_waits(nc):
    # This walrus build rejects instructions carrying more than one sync-wait.
    # Hoist extra waits onto same-engine NoOps placed immediately before the
    # instruction (engine streams preserve relative order within a block).
    for f in nc.m.functions:
        for bb in f.blocks:
            new = []
            for inst in bb.instructions:
                si = inst.sync_info
                if si is not None and si.on_wait and len(si.on_wait) > 1:
                    waits = list(si.on_wait)
                    for w in waits[:-1]:
                        nop = mybir.InstNoOp(
                            name=nc.get_next_instruction_name(),
                            engine=inst.engine,
                            bass_nofuse=True,
                            sync_info=mybir.SyncInfo(on_wait=[w], on_update=[]),
                        )
                        new.append(nop)
                    si.on_wait = [waits[-1]]
                new.append(inst)
            bb.instructions[:] = new


def build_nc():
    nc = bass.Bass()
    x_d = nc.declare_dram_parameter("x", [C, N], FP32, isOutput=False)
    g_d = nc.declare_dram_parameter("gamma", [1, 1], FP32, isOutput=False)
    o_d = nc.declare_dram_parameter("out", [C, N], FP16, isOutput=True)

    # Clear kernel semaphores at START (idle window) instead of paying the
    # expensive teardown clear+barrier at the end (STRIP_TAIL above).  The
    # x-load semaphores below are inside the cleared range, so a re-run of
    # the NEFF starts from zero again.  (Dropping this hangs the device —
    # semaphore state persists across NEFF loads.)
    from concourse.bass import compact_to_ranges

    for sem_range in compact_to_ranges(
        [sem for sem in nc._kernel_sem_range if sem not in nc.barrier_sems]
    ):
        nc.gpsimd.sem_clear(sem_range)
    nc._nrt_pseudo_barrier()

    # Early x loads: raw DMAs issued before the TileContext so the 8MB input
    # streams during the framework prologue.  One [128,1024] chunk per
    # (super-group gg, row-block ci), one completion semaphore each; issue
    # engines are spread so every gg=0 chunk is issued within ~700ns.
    xf = [nc.alloc_sbuf_tensor(f"xraw{ci}", [P, N], FP32) for ci in range(CT)]
    xsem = [[nc.alloc_semaphore(f"xld{gg}_{ci}") for ci in range(CT)] for gg in range(GG)]
    # All loads go through sync's queue IN CONSUMPTION ORDER: a single
    # dynamic queue shards each transfer across all 16 DMA engines (full
    # ~330GB/s per transfer) and completes transfers strictly in order, so
    # chunk (gg,ci) lands every ~1.6us exactly when the ACT cast FIFO wants
    # it.  Spreading issues across engines instead gives each queue a
    # different drain rate and the cast FIFO head-of-line blocks.
    for gg in range(GG):
        for ci in range(CT):
            rows = slice(ci * P, (ci + 1) * P)
            cols = slice(gg * 1024, (gg + 1) * 1024)
            nc.sync.dma_start(out=xf[ci][:, cols], in_=x_d[rows, cols]).then_inc(
                xsem[gg][ci], 16
            )

    cast_waits = []  # (BassInstruction, gg, ci): xsem waits attached post-scheduling

    with tile.TileContext(nc) as tc:
        with (
            tc.tile_pool(name="singles", bufs=1) as singles,
            tc.tile_pool(name="stage", bufs=4) as stage,
            tc.tile_pool(name="psum_acc", bufs=4, space="PSUM") as psum_acc,
            tc.tile_pool(name="psum_po", bufs=3, space="PSUM") as psum_po,
            tc.tile_pool(name="psum_ptr", bufs=1, space="PSUM") as psum_ptr,
        ):
            # PE warm-up on a dep-free tile, ACT Exp-table preload on a dummy,
            # identity before the gamma broadcast DMA.
            warm8 = singles.tile([P, P], FP8, tag="warm8")
            nc.vector.memset(warm8[:], 1.0)
            for _ in range(20):
                wp = psum_po.tile([P, 512], FP32, tag="po")
                nc.tensor.matmul(
                    wp[:, 0:P], lhsT=warm8[:], rhs=warm8[:], start=True, stop=True
                )
            dume = singles.tile([P, 1], FP32, tag="dume")
            nc.scalar.activation(
                out=dume[:], in_=warm8[:, 0:1], func=mybir.ActivationFunctionType.Exp
            )
            id8 = singles.tile([P, P], FP8, tag="id8")
            make_identity(nc, id8)
            gcol = singles.tile([P, 1], FP32, tag="gamma")
            nc.gpsimd.dma_start(out=gcol[:], in_=g_d[:, :].to_broadcast((P, 1)))

            q8 = singles.tile([P, CT, N], FP8, tag="q8")
            qT = singles.tile([P, KT, 512], FP8, tag="qT")
            e_ps = [
                psum_acc.tile([P, 512], FP32, tag="acc", name=f"e{ci}")
                for ci in range(CT)
            ]

            # Phase A: per super-group, cast to fp8 (ACT + GPSIMD, gated on
            # the raw load sems), transpose 128x128 blocks into qT (plain fp8
            # matmul vs identity -> PSUM fp32 -> DVE copy), accumulate
            # full-width DoubleRow energy matmuls (pairs of 128-chunks).
            for gg in range(GG):
                base = gg * 1024
                for ci in range(CT):
                    # All casts on ACT (~0.71us per half-cast = 1.42us per
                    # chunk < 1.6us chunk arrival cadence): the FIFO tracks
                    # the in-order load stream with no backlog.
                    for half in range(2):
                        cst = nc.scalar.copy(
                            out=q8[:, ci, base + half * 512 : base + (half + 1) * 512],
                            in_=xf[ci][:, base + half * 512 : base + (half + 1) * 512],
                        )
                        cast_waits.append((cst, gg, ci))
                for tt in range(4):  # pairs of 128-chunks within super-group
                    t = gg * 4 + tt
                    for k in (2 * t, 2 * t + 1):
                        pt = psum_po.tile([P, 512], FP32, tag="po")
                        for ci in range(CT):
                            nc.tensor.matmul(
                                pt[:, ci * P : (ci + 1) * P],
                                lhsT=q8[:, ci, k * P : (k + 1) * P],
                                rhs=id8[:],
                                start=True,
                                stop=True,
                            )
                        nc.vector.tensor_copy(out=qT[:, k, :], in_=pt[:])
                    for ci in range(CT):
                        nc.tensor.matmul(
                            e_ps[ci][:],
                            lhsT=qT[:, 2 * t : 2 * t + 2, ci * P : (ci + 1) * P],
                            rhs=qT[:, 2 * t : 2 * t + 2, :],
                            start=(t == 0),
                            stop=(t == KT // 2 - 1),
                            perf_mode=DR,
                        )

            # Softmax per ci: row min (DVE), exp with fp8 out + row-sum
            # accumulator (ACT), 1/Z and gamma/Z (DVE small).  attT via plain
            # fp8 matmul transposes + scalar copies into EXPT.  The gamma/Z
            # row scale is NOT applied to EXPQ; it rides in bias2 and is
            # applied by the epilogue adds.  ci=0 is the only chain on the
            # critical path (the others hide under att@q), so its min/exp run
            # in halves to shorten the serial chain.
            mcol = singles.tile([P, CT], FP32, tag="m")
            mh = singles.tile([P, 2], FP32, tag="mh")
            zcol = singles.tile([P, CT], FP32, tag="z")
            zh = singles.tile([P, 2], FP32, tag="zh")
            lnz = singles.tile([P, CT], FP32, tag="lnz")
            bias2 = singles.tile([P, CT], FP32, tag="bias2")
            EXPQ = singles.tile([P, CT, 512], FP8, tag="EXPQ")
            EXPT = singles.tile([P, CT, 512], FP8, tag="EXPT")

            def softmax_head(ci):
                cs = slice(ci, ci + 1)
                if ci == 0:
                    nc.vector.tensor_reduce(
                        out=mh[:, 0:1], in_=e_ps[0][:, 0:256],
                        axis=mybir.AxisListType.X, op=MIN,
                    )
                    nc.vector.tensor_reduce(
                        out=mh[:, 1:2], in_=e_ps[0][:, 256:512],
                        axis=mybir.AxisListType.X, op=MIN,
                    )
                    nc.vector.tensor_tensor(
                        out=mcol[:, 0:1], in0=mh[:, 0:1], in1=mh[:, 1:2], op=MIN
                    )
                else:
                    nc.vector.tensor_reduce(
                        out=mcol[:, cs], in_=e_ps[ci][:],
                        axis=mybir.AxisListType.X, op=MIN,
                    )

            def softmax_tail(ci):
                cs = slice(ci, ci + 1)
                if ci == 0:
                    for half in range(2):
                        nc.scalar.activation(
                            out=EXPQ[:, 0, half * 256 : (half + 1) * 256],
                            in_=e_ps[0][:, half * 256 : (half + 1) * 256],
                            func=mybir.ActivationFunctionType.Exp,
                            bias=mcol[:, 0:1],
                            scale=-1.0,
                            accum_out=zh[:, half : half + 1],
                        )
                    nc.vector.tensor_add(
                        out=zcol[:, 0:1], in0=zh[:, 0:1], in1=zh[:, 1:2]
                    )
                else:
                    nc.scalar.activation(
                        out=EXPQ[:, ci, :],
                        in_=e_ps[ci][:],
                        func=mybir.ActivationFunctionType.Exp,
                        bias=mcol[:, cs],
                        scale=-1.0,
                        accum_out=zcol[:, cs],
                    )
                nc.vector.reciprocal(out=lnz[:, cs], in_=zcol[:, cs])
                nc.vector.tensor_mul(out=bias2[:, cs], in0=lnz[:, cs], in1=gcol[:])
                for dj in range(CT):
                    ptx = psum_ptr.tile([P, P], FP32, tag="ptr")
                    nc.tensor.matmul(
                        ptx[:],
                        lhsT=EXPQ[:, ci, dj * P : (dj + 1) * P],
                        rhs=id8[:],
                        start=True,
                        stop=True,
                    )
                    nc.scalar.copy(
                        out=EXPT[:, dj, ci * P : (ci + 1) * P], in_=ptx[:]
                    )

            def attq(ci):
                # att@q (DoubleRow, K=512 via two K=256 groups) + fused
                # epilogue add out = po * (gamma/Z_c) + x, fp16 store.
                for nh in range(2):
                    osb = stage.tile([P, 2048], FP16, tag="osb")
                    for sub in range(4):
                        nj = nh * 4 + sub
                        po = psum_po.tile([P, 512], FP32, tag="po")
                        for j in range(2):
                            nc.tensor.matmul(
                                po[:],
                                lhsT=EXPT[:, 2 * j : 2 * j + 2, ci * P : (ci + 1) * P],
                                rhs=q8[:, 2 * j : 2 * j + 2, nj * 512 : (nj + 1) * 512],
                                start=(j == 0),
                                stop=(j == 1),
                                perf_mode=DR,
                            )
                        if sub < 3:
                            nc.vector.scalar_tensor_tensor(
                                out=osb[:, sub * 512 : (sub + 1) * 512],
                                in0=po[:],
                                scalar=bias2[:, ci : ci + 1],
                                in1=xf[ci][:, nj * 512 : (nj + 1) * 512],
                                op0=mybir.AluOpType.mult,
                                op1=mybir.AluOpType.add,
                            )
                        else:
                            tmp = stage.tile([P, 512], FP32, tag="tmp")
                            nc.scalar.mul(
                                out=tmp[:], in_=po[:], mul=bias2[:, ci : ci + 1]
                            )
                            nc.gpsimd.tensor_add(
                                out=osb[:, sub * 512 : (sub + 1) * 512],
                                in0=tmp[:],
                                in1=xf[ci][:, nj * 512 : (nj + 1) * 512],
                            )
                    nc.sync.dma_start(
                        out=o_d[ci * P : (ci + 1) * P, nh * 2048 : (nh + 1) * 2048],
                        in_=osb[:],
                    )

            for ci in range(CT):
                softmax_head(ci)
            for ci in range(CT):
                softmax_tail(ci)
                attq(ci)

    # The raw-load gating is invisible to the tile scheduler (its deadlock
    # simulator would stall on semaphores no in-context instruction bumps),
    # so attach the waits only after scheduling has run.
    for cst, gg, ci in cast_waits:
        cst.wait_op(xsem[gg][ci], 16, "sem-ge")
    _legalize_sync_waits(nc)
    return nc


def make_in_maps(x, gamma):
    x = np.ascontiguousarray(np.asarray(x, dtype=np.float32)).reshape(B, C, N)
    g = np.ascontiguousarray(np.asarray(gamma, dtype=np.float32)).reshape(1, 1)
    return [{"x": x[i], "gamma": g} for i in range(B)]


def kernel(x, y=None, gamma=None, **_ignored):
    from concourse.bass_utils import run_bass_kernel_spmd

    nc = build_nc()
    in_maps = make_in_maps(x, gamma)
    res = run_bass_kernel_spmd(nc, in_maps, list(range(B)))
    out = np.stack([np.asarray(res.results[i]["out"]) for i in range(B)])
    return out.reshape(B, C, 64, 64).astype(np.float32)
